# revision 20
# baseline (speedup 1.0000x reference)
"""Expert-parallel MoE SwiGLU kernel for 8 Trainium2 NeuronCores.

Problem: N=4096 tokens, top-2 of E=8 experts, H=2048, I=1408, fp32.

Strategy (load-balanced expert parallel, S segments per core):
  - Host-side dispatch: sort routed (token, k) slots by expert, then pack
    them into 8*S fixed-capacity column slots -- each core gets S slots
    with caps caps[0] >= caps[1] >= ... (same on every core, SPMD), each
    slot holding tokens of a single expert.  Caps are solved per routing
    instance to minimize sum(caps) subject to a feasible expert->slots
    assignment (with 3 segments the top-k counts pack to ~1040 columns
    vs 1064 for 2 segments vs 1152 for naive one-expert-per-core).
  - Device (same SPMD program on all 8 cores): for each segment s
        y^T[:, seg_s] = Wd_s @ (silu(Wg_s @ x^T[:, seg_s]) *
                                (Wu_s @ x^T[:, seg_s]))
    entirely in [feature, token] layout, bf16 matmuls (single-pass PE
    rate; abs-max rel err vs fp32 reference ~5e-3).
  - Host-side combine: weighted scatter-add of per-slot outputs back to
    the [N, H] output.

Schedule notes (from perfetto traces):
  - DMAs have ~2-2.5us fixed DGE latency + ~5ns/descriptor; front
    pieces must be few and large, and the final output DMA is the tail
    floor.
  - The PE HAM clock-gate needs ~3.4us of sustained busy-ness to lift
    the 1.2GHz cold throttle: dummy warm-up matmuls run while the first
    weight/x pieces stream in.
  - Phase-1 front is HBM-bound: smallest segment is processed first
    (least front bytes); weight streams for later i / later segments
    are dep-paced so they don't steal front bandwidth.
"""

import numpy as np

import concourse.bass as bass
import concourse.tile as tile
from concourse import bacc, mybir
from concourse import bass_utils
from concourse.tile import add_dep_helper

N, K, E, H, I = 4096, 2, 8, 2048, 1408
P = 128
HCH = H // P   # 16 chunks over hidden dim
ICH = I // P   # 11 chunks over intermediate dim
F32 = mybir.dt.float32
F32R = mybir.dt.float32r
BF16 = mybir.dt.bfloat16
LET = "ABC"


def _r8(v):
    return -(-int(v) // 8) * 8


def _split_cols(width, base=0):
    """Split width into n<=ceil(width/512) chunks each <=512, mult-of-8-ish.
    Returns [(abs_off, w), ...] offset by `base`."""
    n = -(-width // 512)
    out, off = [], 0
    for j in range(n):
        w = _r8((width - off) // (n - j)) if j < n - 1 else width - off
        out.append((base + off, w))
        off += w
    assert off == width and all(w <= 512 for _, w in out)
    return out


def _build(caps, xdt=BF16, wdt=BF16, hdt=BF16):
    """Build + compile the per-core S-segment SwiGLU kernel."""
    S = len(caps)
    C = sum(caps)
    offs = [sum(caps[:s]) for s in range(S)]
    segch = [_split_cols(caps[s], offs[s]) for s in range(S)]
    p1order = sorted(range(S), key=lambda s: caps[s])  # smallest seg first
    f = p1order[0]
    assert offs[f] + caps[f] == C  # smallest seg sits at the column end
    nch_all = sum(len(c) for c in segch)
    assert 2 * nch_all + 1 <= 8        # phase-1 psg/psu banks + warm-up
    assert 2 * nch_all + 1 <= 8        # phase-2 (bufs=2 per chunk) + warm

    nc = bacc.Bacc("TRN2", target_bir_lowering=False, debug=False,
                   enable_asserts=False, num_devices=E)

    xT = nc.dram_tensor("xT", [H, C], xdt, kind="ExternalInput")
    # weights come host-pre-tiled so every DMA line is contiguous:
    # wg[i, p, h*128+j] = Wg[e][i*128+j, h*128+p]  (lhsT tiles back to back)
    wseg = []
    for s in range(S):
        L = LET[s]
        wseg.append((
            nc.dram_tensor(f"wg{L}", [ICH, P, H], wdt, kind="ExternalInput"),
            nc.dram_tensor(f"wu{L}", [ICH, P, H], wdt, kind="ExternalInput"),
            nc.dram_tensor(f"wd{L}", [HCH, P, I], wdt, kind="ExternalInput"),
        ))
    outT = nc.dram_tensor("outT", [H, C], BF16, kind="ExternalOutput")

    x_r = xT.ap().rearrange("(ho p) c -> p ho c", p=P)      # [128, 16, C]
    out_r = outT.ap().rearrange("(ho p) c -> p ho c", p=P)  # [128, 16, C]

    with tile.TileContext(nc) as tc:
        with (
            tc.tile_pool(name="xpool", bufs=1) as xpool,
            tc.tile_pool(name="hpool", bufs=1) as hpool,
            tc.tile_pool(name="wpool0", bufs=4) as wpool0,
            tc.tile_pool(name="wpool1", bufs=3) as wpool1,
            tc.tile_pool(name="wpool2", bufs=3) as wpool2,
            tc.tile_pool(name="dpool", bufs=3) as dpool,
            tc.tile_pool(name="opool", bufs=2) as opool,
            tc.tile_pool(name="wps", bufs=1, space="PSUM") as wps,
        ):
            # weight pool per segment: first-processed gets 3 buffers
            wpools = {}
            for kseg, seg in enumerate(p1order):
                wpools[seg] = (wpool0, wpool1, wpool2)[kseg]

            x_sb = xpool.tile([P, HCH, C], xdt)
            hid_sb = hpool.tile([P, ICH, C], hdt)

            # ---- PE warm-up ---------------------------------------------
            # HAM clock-gates the PE to 1.2 GHz until it sees ~3.4us of
            # sustained busy-ness; the front leaves the PE idle until the
            # first pieces land (~10us).  Dummy matmuls (no deps -> hoisted
            # to t~7us when the PE queue unblocks) bridge the window.
            NWARM = 30
            warm_sb = xpool.tile([P, 2 * P], xdt, name="warm_sb")
            nc.vector.memset(warm_sb[:], 0.0)
            warm_ps = wps.tile([P, P], F32, name="warm_ps")
            for _ in range(NWARM):
                nc.tensor.matmul(warm_ps[:], warm_sb[:, 0:P],
                                 warm_sb[:, P:2 * P], start=True,
                                 stop=True)

            # ---- front: first segment's i=0 weights + x, interleaved ----
            # Delivery order == issue order per ring; pieces are issued in
            # exact i=0 consumption order (h-spans), weights on the sync
            # ring, x on the scalar ring, so the first matmuls can chase
            # the stream (~10.6us) instead of waiting for the whole front.
            fg_r = wseg[f][0].ap()
            fu_r = wseg[f][1].ap()
            w0 = wpools[f].tile([P, 2, H], wdt, tag=f"w{LET[f]}",
                                name=f"w{LET[f]}_0")
            xb = {}
            for k, (h0, h1) in enumerate(((0, 2), (2, 6), (6, 10), (10, 16))):
                nc.sync.dma_start(w0[:, 0, h0 * P:h1 * P],
                                  fg_r[0][:, h0 * P:h1 * P])
                nc.sync.dma_start(w0[:, 1, h0 * P:h1 * P],
                                  fu_r[0][:, h0 * P:h1 * P])
                xb[k] = nc.scalar.dma_start(x_sb[:, h0:h1, offs[f]:C],
                                            x_r[:, h0:h1, offs[f]:C])

            # ---- phase 1: gate/up + SwiGLU per segment -> hidden^T -----
            # The Tile scheduler hoists any DMA with no dependencies to the
            # start, so every first-buffer DMA (before pool WAR pacing kicks
            # in) is explicitly gated to keep the front HBM window clean.
            wd1 = {s: {} for s in range(S)}  # per-seg weight-u DMAs (pacing)
            wlast = []                       # last seg's DMAs, pace wd below
            with tc.tile_pool(name="ps1", bufs=1, space="PSUM") as ps1:
                for kseg, seg in enumerate(p1order):
                    ch = segch[seg]
                    stag = LET[seg]
                    wg_r, wu_r = wseg[seg][0].ap(), wseg[seg][1].ap()
                    for i in range(ICH):
                        if kseg == 0 and i == 0:
                            w_sb = w0
                        else:
                            w_sb = wpools[seg].tile([P, 2, H], wdt,
                                                    tag=f"w{stag}",
                                                    name=f"w{stag}_{i}")
                            d0 = nc.sync.dma_start(w_sb[:, 0], wg_r[i])
                            d1 = nc.sync.dma_start(w_sb[:, 1], wu_r[i])
                            if kseg == 0 and i == 1:
                                # stream i=1 weights concurrent with the
                                # last x pieces so i=1 starts stall-free
                                add_dep_helper(d0.ins, xb[2].ins,
                                               reason="pace w1 into front tail")
                                add_dep_helper(d1.ins, xb[2].ins,
                                               reason="pace w1 into front tail")
                            if kseg == 0 and i in (2, 3):
                                # fresh buffers (bufs=4, no WAR) -- gate
                                # them so they are not hoisted into the
                                # front; chain each behind the previous i
                                add_dep_helper(d0.ins, wd1[seg][i - 1].ins,
                                               reason="pace behind prev i")
                                add_dep_helper(d1.ins, wd1[seg][i - 1].ins,
                                               reason="pace behind prev i")
                            if kseg > 0 and i < 3:
                                # first buffers have no pool predecessor
                                prev = p1order[kseg - 1]
                                add_dep_helper(d0.ins, wd1[prev][4 + i].ins,
                                               reason="pace behind prev seg")
                                add_dep_helper(d1.ins, wd1[prev][4 + i].ins,
                                               reason="pace behind prev seg")
                            wd1[seg][i] = d1
                            if kseg == S - 1:
                                wlast.append(d1)
                        if kseg == 0 and i == 3:
                            # x for the remaining segments, held behind the
                            # early first-seg stream (needed only when the
                            # next segment's phase 1 starts)
                            xa0 = nc.scalar.dma_start(x_sb[:, 0:8, 0:offs[f]],
                                                      x_r[:, 0:8, 0:offs[f]])
                            add_dep_helper(xa0.ins, wd1[f][1].ins,
                                           reason="yield front BW")
                            xa1 = nc.scalar.dma_start(x_sb[:, 8:16, 0:offs[f]],
                                                      x_r[:, 8:16, 0:offs[f]])
                            add_dep_helper(xa1.ins, wd1[f][2].ins,
                                           reason="yield front BW")
                        ps_g = [
                            ps1.tile([P, cw], F32, name=f"psg{stag}_{i}_{n}",
                                     tag=f"psg{stag}{n}")
                            for n, (c0, cw) in enumerate(ch)
                        ]
                        ps_u = [
                            ps1.tile([P, cw], F32, name=f"psu{stag}_{i}_{n}",
                                     tag=f"psu{stag}{n}")
                            for n, (c0, cw) in enumerate(ch)
                        ]
                        # Only the first segment's i=0 interleaves gate/up
                        # per h (widens the front x-deadline to the whole
                        # i=0 span); all other groups keep m-outer order so
                        # the gate psum stop lands mid-group and the silu
                        # read never stalls the next group's first matmul.
                        if kseg == 0 and i == 0:
                            mh_iter = [(h, m) for h in range(HCH)
                                       for m in (0, 1)]
                        else:
                            mh_iter = [(h, m) for m in (0, 1)
                                       for h in range(HCH)]
                        last_group = (kseg == S - 1 and i == ICH - 1)
                        for h, m in mh_iter:
                            ps = ps_g if m == 0 else ps_u
                            lhsT = w_sb[:, m, h * P:(h + 1) * P]
                            for n, (c0, cw) in enumerate(ch):
                                if last_group and m == 1:
                                    continue      # emitted chunk-major below
                                nc.tensor.matmul(
                                    ps[n][:],
                                    lhsT,
                                    x_sb[:, h, c0:c0 + cw],
                                    start=(h == 0),
                                    stop=(h == HCH - 1),
                                )
                        if last_group:
                            # final group: run the up sweep chunk-major so
                            # the first chunk's psum stop (and its mul) land
                            # early -- shortens the phase-2 entry wait on
                            # the ps1 pool close.
                            for n, (c0, cw) in enumerate(ch):
                                for h in range(HCH):
                                    nc.tensor.matmul(
                                        ps_u[n][:],
                                        w_sb[:, 1, h * P:(h + 1) * P],
                                        x_sb[:, h, c0:c0 + cw],
                                        start=(h == 0),
                                        stop=(h == HCH - 1),
                                    )
                        for n, (c0, cw) in enumerate(ch):
                            hs = hid_sb[:, i, c0:c0 + cw]
                            nc.scalar.activation(
                                out=hs, in_=ps_g[n][:],
                                func=mybir.ActivationFunctionType.Silu,
                            )
                            nc.vector.tensor_mul(out=hs, in0=hs, in1=ps_u[n][:])

            # ---- phase 2: down projection -> out^T [H, C] --------------
            # The h rows are processed as column-clipped "jobs".  h=HCH-1
            # is split: most of its columns run as the SECOND job (their
            # output ships ~70us early), and only a small TAILW-column
            # piece runs last -- after the final matmul only one small
            # copy + one small DMA (on the otherwise-idle scalar ring)
            # remain instead of a full C-column row (~2.5us DGE latency
            # per DMA).
            TAILW = 160
            assert caps[f] > TAILW
            jobs = ([(0, 0, C), (HCH - 1, 0, C - TAILW)]
                    + [(h, 0, C) for h in range(1, HCH - 1)]
                    + [(HCH - 1, C - TAILW, C)])
            # first-processed segment's chunks first: their psum chains
            # stop earliest, so their copies overlap remaining matmuls
            p2ch_full = [(c0, cw, s) for s in p1order for (c0, cw) in segch[s]]
            with tc.tile_pool(name="ps2", bufs=2, space="PSUM") as ps2:
                for jn, (h, lo, hi) in enumerate(jobs):
                    final = jn == len(jobs) - 1
                    wd_sb = {}
                    for seg in range(S):
                        wd_r = wseg[seg][2].ap()
                        t = dpool.tile([P, I], wdt, tag=f"wd{LET[seg]}")
                        dd = nc.gpsimd.dma_start(t[:], wd_r[h])
                        # SWDGE queues are parallel, so every wd DMA must be
                        # gated individually or it floods the front/phase-1
                        # weight stream; hold them until mid-way through the
                        # last segment's weight stream (SWDGE queues are slow
                        # -- ~12us per 360KB tile -- so phase 2's first jobs
                        # need their wd tiles in flight well before the
                        # phase-1/phase-2 transition).
                        add_dep_helper(dd.ins, wlast[4].ins,
                                       reason="pace wd behind phase-1 weights")
                        wd_sb[seg] = t
                    # clip chunks to this job's column window
                    p2ch = []
                    for c0, cw, s in p2ch_full:
                        l2, h2 = max(c0, lo), min(c0 + cw, hi)
                        if h2 > l2:
                            p2ch.append((l2, h2 - l2, s))
                    ps_d = [
                        ps2.tile([P, cw], F32, name=f"psd_{jn}_{n}",
                                 tag=f"psd{n}")
                        for n, (c0, cw, s) in enumerate(p2ch)
                    ]
                    for i in range(ICH):
                        for n, (c0, cw, s) in enumerate(p2ch):
                            nc.tensor.matmul(
                                ps_d[n][:],
                                wd_sb[s][:, i * P:(i + 1) * P],
                                hid_sb[:, i, c0:c0 + cw],
                                start=(i == 0),
                                stop=(i == ICH - 1),
                            )
                    o_sb = opool.tile([P, C], BF16, tag="o")
                    for n, (c0, cw, s) in enumerate(p2ch):
                        if n == 0 and not final:
                            nc.scalar.activation(
                                out=o_sb[:, c0:c0 + cw], in_=ps_d[n][:],
                                func=mybir.ActivationFunctionType.Copy,
                            )
                        else:
                            nc.vector.tensor_copy(o_sb[:, c0:c0 + cw],
                                                  ps_d[n][:])
                    ring = nc.scalar if final else nc.sync
                    ring.dma_start(out_r[:, h, lo:hi], o_sb[:, lo:hi])

    nc.compile()
    return nc


_NC_CACHE = {}

# compute dtype config: "f32r" (FP22 single-pass, ~3e-4 rel err) or "bf16"
DTYPES = {
    "f32r": (F32R, F32R, F32R),
    "bf16": (BF16, BF16, BF16),
}
import os
CONFIG = os.environ.get("MOE_KERNEL_CONFIG", "bf16")


def _get_nc(caps):
    key = (tuple(caps), CONFIG)
    if key not in _NC_CACHE:
        _NC_CACHE[key] = _build(tuple(caps), *DTYPES[CONFIG])
    return _NC_CACHE[key]


def _solve_caps2(counts):
    """2-segment caps (a, b) + expert->slots plan (previous scheme)."""
    counts = np.asarray(counts)
    order = np.argsort(-counts, kind="stable")
    c = counts[order]
    best = None
    for m2 in (0, 2, 4, 6, 8):          # |M| = experts using one A + one B
        g = (8 - m2) // 2               # |A2| = |B2|
        if g == 0:
            Cc = _r8(c[0])
            aa = _r8(-(-c[0] // 2))
            bb = Cc - aa
        else:
            maxA2 = c[:g].max()
            maxB2 = c[g + m2:].max()
            aa = _r8(-(-maxA2 // 2))
            bb = _r8(-(-maxB2 // 2))
            if m2:
                bb = max(bb, _r8(c[g:g + m2].max() - aa))
            Cc = aa + bb
        if bb < 1 or aa < bb:
            continue
        if best is None or Cc < best[0]:
            best = (Cc, aa, bb, m2, g)
    _, a, b, m2, g = best
    a_slots = [(core, 0) for core in range(8)]
    b_slots = [(core, 1) for core in range(8)]
    plan = []
    for j, e in enumerate(order):
        if j < g:                        # heavy: two A-slots
            plan.append((e, [a_slots.pop(0), a_slots.pop(0)]))
        elif j < g + m2:                 # middling: A + B
            plan.append((e, [a_slots.pop(0), b_slots.pop(0)]))
        else:                            # light: two B-slots
            plan.append((e, [b_slots.pop(0), b_slots.pop(0)]))
    return (a, b), plan


def _solve_caps3(counts):
    """3-segment caps (a,b,c) + plan, or None.  Exactly 3 slots/expert."""
    import itertools
    counts = np.asarray(counts)
    order = np.argsort(-counts, kind="stable")
    cs = [int(v) for v in counts[order]]
    cmax = cs[0]

    def feasible(caps):
        triples = []
        for combo in itertools.combinations_with_replacement(range(3), 3):
            ssum = sum(caps[i] for i in combo)
            use = [0, 0, 0]
            for i in combo:
                use[i] += 1
            triples.append((ssum, tuple(use)))
        opts = []
        for cnt in cs:
            o = [u for ssum, u in triples if ssum >= cnt]
            if not o:
                return None
            opts.append(o)
        sol = []

        def dfs(e, ra, rb, rc):
            if e == 8:
                return True
            for u in opts[e]:
                if u[0] <= ra and u[1] <= rb and u[2] <= rc:
                    sol.append(u)
                    if dfs(e + 1, ra - u[0], rb - u[1], rc - u[2]):
                        return True
                    sol.pop()
            return False
        return sol if dfs(0, 8, 8, 8) else None

    for C in range(1024, 1064, 8):
        for a in range(_r8(C // 3), min(520, C - 384) + 1, 8):
            if 3 * a < cmax and 2 * a + (C - a) // 2 < cmax:
                continue
            for b in range(_r8((C - a + 1) // 2), a + 1, 8):
                c = C - a - b
                if c < 200 or c > b:
                    continue
                if caps_sol := feasible((a, b, c)):
                    caps = (a, b, c)
                    # build plan: pop per-seg core lists
                    free = [list(range(8)) for _ in range(3)]
                    plan = []
                    for j, e in enumerate(order):
                        use = caps_sol[j]
                        slots = []
                        for s in range(3):
                            for _ in range(use[s]):
                                slots.append((free[s].pop(0), s))
                        plan.append((e, slots))
                    return caps, plan
    return None


def _solve_caps(counts):
    caps2, plan2 = _solve_caps2(counts)
    r3 = _solve_caps3(counts)
    if r3 is not None and sum(r3[0]) < sum(caps2):
        return r3
    return caps2, plan2


def kernel(x, topk_ids, topk_weight, Wg, Wu, Wd):
    x = np.asarray(x, dtype=np.float32)
    topk_ids = np.asarray(topk_ids)
    topk_weight = np.asarray(topk_weight, dtype=np.float32)

    # ---- host-side dispatch (the all-to-all by topk_ids)
    flat = topk_ids.reshape(-1).astype(np.int64)
    order = np.argsort(flat, kind="stable")
    counts = np.bincount(flat, minlength=E)
    toks = order // K          # token index per sorted slot
    ks = order % K             # which of the top-k slots
    bounds = np.cumsum(counts)
    starts = bounds - counts

    caps, plan = _solve_caps(counts)
    S = len(caps)
    offs = [sum(caps[:s]) for s in range(S)]
    C = sum(caps)
    nc = _get_nc(caps)

    import ml_dtypes
    xdt, wdt, _ = DTYPES[CONFIG]
    np_x = ml_dtypes.bfloat16 if xdt == BF16 else np.float32
    np_w = ml_dtypes.bfloat16 if wdt == BF16 else np.float32

    def pack_gu(w):  # [I, H] -> [ICH, P, H]; out[i, p, h*128+j] = w[i*128+j, h*128+p]
        v = np.asarray(w, np.float32).reshape(ICH, P, HCH, P)       # [i, j, h, p]
        return np.ascontiguousarray(
            v.transpose(0, 3, 2, 1).astype(np_w)).reshape(ICH, P, H)

    def pack_d(w):   # [H, I] -> [HCH, P, I]; out[h, p, i*128+j] = w[h*128+j, i*128+p]
        v = np.asarray(w, np.float32).reshape(HCH, P, ICH, P)       # [h, j, i, p]
        return np.ascontiguousarray(
            v.transpose(0, 3, 2, 1).astype(np_w)).reshape(HCH, P, I)

    packed = {}

    def get_packed(e):
        if e not in packed:
            packed[e] = (pack_gu(Wg[e]), pack_gu(Wu[e]), pack_d(Wd[e]))
        return packed[e]

    # fill slots with each expert's routed tokens, in slot order
    core_slots = [[None] * S for _ in range(E)]   # [(expert, toks, ks)]
    for e, slots in plan:
        te = toks[starts[e]:bounds[e]]
        ke = ks[starts[e]:bounds[e]]
        off = 0
        for core, seg in slots:
            cap = caps[seg]
            n = min(cap, len(te) - off)
            n = max(n, 0)
            core_slots[core][seg] = (e, te[off:off + n], ke[off:off + n])
            off += n
        assert off == len(te), (e, off, len(te))

    in_maps = []
    for core in range(E):
        xT_c = np.zeros((H, C), np_x)
        m = {"xT": xT_c}
        for seg in range(S):
            e, te, ke = core_slots[core][seg]
            base = offs[seg]
            if len(te):
                xT_c[:, base:base + len(te)] = x[te].T.astype(np_x)
            pg, pu, pd = get_packed(e)
            L = LET[seg]
            m[f"wg{L}"], m[f"wu{L}"], m[f"wd{L}"] = pg, pu, pd
        in_maps.append(m)

    res = bass_utils.run_bass_kernel_spmd(nc, in_maps, core_ids=list(range(E)))

    # ---- host-side combine (weighted scatter-add)
    out = np.zeros((N, H), np.float32)
    for core in range(E):
        yT = np.asarray(res.results[core]["outT"], dtype=np.float32)
        for seg in range(S):
            e, te, ke = core_slots[core][seg]
            if len(te) == 0:
                continue
            base = offs[seg]
            y = yT[:, base:base + len(te)]                # [H, n]
            w = topk_weight[te, ke].astype(np.float32)
            out[te] += (y * w[None, :]).T
    return out
